# revision 6
# baseline (speedup 1.0000x reference)
"""Causal self-attention MLA (GQA, latent kv) kernel for 8 Trainium2 cores.

Sharding: the 8 cores map to (batch b, kv-group g) pairs: core = b*4 + g.
Each core computes, for its batch and its kv head (4 q-heads):
  qT = Wq_g^T x^T (rope), cT = Wc^T x^T, kT = Wk_g^T cT (rope), v = (Wv_g^T cT)^T
  flash attention entirely in the transposed domain:
    ST[k,q] = kT^T qT  (per 128-k-block, causal blocks only)
    PT = exp(SCALE*ST + keybias)       (no max subtraction; logits ~N(0,1))
    yT[d,q] += v[kb]^T PT              (moving = PT -> no transposes anywhere)
    rs[*,q] += ones^T PT               (rowsum replicated across partitions)
    yTn = yT * 1/rs
  out_partial = yTn^T Wo_g  (row-parallel out proj)
Host sums the 4 partials per batch (free w.r.t. HW time).

All matmuls run in float32r (TF32-like, 11-bit mantissa, full PE rate for
moving dim >= 256); accumulation is fp32 in PSUM. End-to-end rel err ~1e-4.
"""
import numpy as np

import concourse.bacc as bacc
import concourse.mybir as mybir
import concourse.tile as tile
from concourse.bass_utils import run_bass_kernel_spmd

B, L, HID = 2, 2048, 2048
NH, NKV, HD = 16, 4, 128
LAT = 512
QPG = NH // NKV            # q heads per kv group = 4
SCALE = float(HD) ** -0.5
ROPE_THETA = 10000.0
P = 128
NT = L // 512              # 4 token chunks of 512
KT = HID // P              # 16 contraction tiles
LT = LAT // P              # 4 latent tiles
TT = L // P                # 16 token tiles of 128

dt = mybir.dt
f32, f32r = dt.float32, dt.float32r

_CACHE = {}


def _build():
    nc = bacc.Bacc("TRN2", target_bir_lowering=False, debug=False)

    xT_d = nc.dram_tensor("xT", [HID, L], f32r, kind="ExternalInput")
    wq_d = nc.dram_tensor("wq", [HID, QPG * HD], f32r, kind="ExternalInput")
    wc_d = nc.dram_tensor("wc", [HID, LAT], f32r, kind="ExternalInput")
    wk_d = nc.dram_tensor("wk", [LAT, HD], f32r, kind="ExternalInput")
    wv_d = nc.dram_tensor("wv", [LAT, HD], f32r, kind="ExternalInput")
    wo_d = nc.dram_tensor("wo", [QPG * HD, HID], f32r, kind="ExternalInput")
    cos_d = nc.dram_tensor("cos2", [P, L], f32r, kind="ExternalInput")
    sin_d = nc.dram_tensor("sin2", [P, L], f32r, kind="ExternalInput")
    neg_d = nc.dram_tensor("negmask", [P, P], f32, kind="ExternalInput")
    ones_d = nc.dram_tensor("onesm", [P, P], f32r, kind="ExternalInput")
    idn_d = nc.dram_tensor("ident", [P, P], f32r, kind="ExternalInput")
    kb_d = nc.dram_tensor("keybias", [P, TT], f32, kind="ExternalInput")
    out_d = nc.dram_tensor("out", [L, HID], f32, kind="ExternalOutput")

    with tile.TileContext(nc) as tc:
        with tc.tile_pool(name="consts", bufs=1) as cp, \
             tc.tile_pool(name="qt", bufs=1) as qtp, \
             tc.tile_pool(name="kt", bufs=1) as ktp, \
             tc.tile_pool(name="vnat", bufs=1) as vnp:

            cos_t = cp.tile([P, L], f32r)
            sin_t = cp.tile([P, L], f32r)
            neg_t = cp.tile([P, P], f32)
            ones_t = cp.tile([P, P], f32r)
            idn_t = cp.tile([P, P], f32r)
            kbias_t = cp.tile([P, TT], f32)
            nc.sync.dma_start(cos_t[:], cos_d[:])
            nc.sync.dma_start(sin_t[:], sin_d[:])
            nc.sync.dma_start(neg_t[:], neg_d[:])
            nc.sync.dma_start(ones_t[:], ones_d[:])
            nc.sync.dma_start(idn_t[:], idn_d[:])
            nc.sync.dma_start(kbias_t[:], kb_d[:])

            qT = qtp.tile([P, QPG, L], f32r)     # per-head qT, roped in place
            kT = ktp.tile([P, L], f32r)          # kv-group kT, roped in place
            v_sb = vnp.tile([P, TT, HD], f32r)   # v natural [k, tile, d]

            # ---------------- phase 1: projections ----------------
            with tc.tile_pool(name="wqc", bufs=1) as wp, \
                 tc.tile_pool(name="xt", bufs=6) as xp, \
                 tc.tile_pool(name="ct", bufs=10) as ctp, \
                 tc.tile_pool(name="vt", bufs=2) as vtp, \
                 tc.tile_pool(name="rtmp", bufs=8) as rtp, \
                 tc.tile_pool(name="ps1", bufs=8, space="PSUM") as ps1:

                wq_t = wp.tile([P, KT, QPG * HD], f32r)
                wc_t = wp.tile([P, KT, LAT], f32r)
                wk_t = wp.tile([P, LT, HD], f32r)
                wv_t = wp.tile([P, LT, HD], f32r)
                nc.sync.dma_start(
                    wq_t[:], wq_d.rearrange("(kt p) m -> p kt m", p=P))
                nc.sync.dma_start(
                    wc_t[:], wc_d.rearrange("(kt p) m -> p kt m", p=P))
                nc.sync.dma_start(
                    wk_t[:], wk_d.rearrange("(lt p) m -> p lt m", p=P))
                nc.sync.dma_start(
                    wv_t[:], wv_d.rearrange("(lt p) m -> p lt m", p=P))

                def rope_chunk(dst, tc_idx):
                    """In-place rope of dst[:, c0:c1] ([128, 512] f32r)."""
                    c0, c1 = tc_idx * 512, (tc_idx + 1) * 512
                    t1c = rtp.tile([64, 512], f32, tag="rt")
                    t1s = rtp.tile([64, 512], f32, tag="rt")
                    t2c = rtp.tile([64, 512], f32, tag="rt")
                    t2s = rtp.tile([64, 512], f32, tag="rt")
                    nc.vector.tensor_mul(t1c[:], dst[0:64, c0:c1], cos_t[0:64, c0:c1])
                    nc.vector.tensor_mul(t1s[:], dst[0:64, c0:c1], sin_t[0:64, c0:c1])
                    nc.vector.tensor_mul(t2c[:], dst[64:128, c0:c1], cos_t[64:128, c0:c1])
                    nc.vector.tensor_mul(t2s[:], dst[64:128, c0:c1], sin_t[64:128, c0:c1])
                    nc.vector.tensor_sub(dst[0:64, c0:c1], t1c[:], t2s[:])
                    nc.vector.tensor_add(dst[64:128, c0:c1], t2c[:], t1s[:])

                for t in range(NT):
                    c0, c1 = t * 512, (t + 1) * 512
                    qps = [ps1.tile([P, 512], f32, tag="ps1", name=f"qps{t}_{i}")
                           for i in range(QPG)]
                    cps = [ps1.tile([P, 512], f32, tag="ps1", name=f"cps{t}_{i}")
                           for i in range(LT)]
                    for kt in range(KT):
                        xt = xp.tile([P, 512], f32r, tag="xt")
                        nc.sync.dma_start(xt[:], xT_d[kt * P:(kt + 1) * P, c0:c1])
                        st, sp = (kt == 0), (kt == KT - 1)
                        for h in range(QPG):
                            nc.tensor.matmul(
                                qps[h][:], wq_t[:, kt, h * HD:(h + 1) * HD],
                                xt[:], start=st, stop=sp)
                        for l in range(LT):
                            nc.tensor.matmul(
                                cps[l][:], wc_t[:, kt, l * P:(l + 1) * P],
                                xt[:], start=st, stop=sp)
                    c_tiles = []
                    for l in range(LT):
                        ct = ctp.tile([P, 512], f32r, tag="ct")
                        nc.vector.tensor_copy(ct[:], cps[l][:])
                        c_tiles.append(ct)
                    for h in range(QPG):
                        nc.vector.tensor_copy(qT[:, h, c0:c1], qps[h][:])
                        rope_chunk(qT[:, h, :], t)

                    # kT / vT for this token chunk (contraction over LAT)
                    kps = ps1.tile([P, 512], f32, tag="ps1")
                    vps = ps1.tile([P, 512], f32, tag="ps1")
                    for l in range(LT):
                        nc.tensor.matmul(kps[:], wk_t[:, l, :], c_tiles[l][:],
                                         start=(l == 0), stop=(l == LT - 1))
                    for l in range(LT):
                        nc.tensor.matmul(vps[:], wv_t[:, l, :], c_tiles[l][:],
                                         start=(l == 0), stop=(l == LT - 1))
                    nc.vector.tensor_copy(kT[:, c0:c1], kps[:])
                    rope_chunk(kT, t)
                    vt = vtp.tile([P, 512], f32r, tag="vt")
                    nc.vector.tensor_copy(vt[:], vps[:])
                    for s in range(4):
                        tp = ps1.tile([P, P], f32r, tag="ps1")
                        nc.tensor.transpose(tp[:], vt[:, s * P:(s + 1) * P], idn_t[:])
                        nc.vector.tensor_copy(v_sb[:, t * 4 + s, :], tp[:])

            # ---------------- phases 2+3: attention + out proj ----------------
            with tc.tile_pool(name="wo", bufs=1) as wop, \
                 tc.tile_pool(name="yt", bufs=1) as ytp, \
                 tc.tile_pool(name="pt", bufs=4) as ptp, \
                 tc.tile_pool(name="rc", bufs=2) as rcp, \
                 tc.tile_pool(name="ot", bufs=4) as otp, \
                 tc.tile_pool(name="ps_st", bufs=2, space="PSUM") as ps_st, \
                 tc.tile_pool(name="ps_y", bufs=2, space="PSUM") as ps_y, \
                 tc.tile_pool(name="ps_rs", bufs=2, space="PSUM") as ps_rs, \
                 tc.tile_pool(name="ps_o", bufs=2, space="PSUM") as ps_o:

                wo_t = wop.tile([P, QPG, HID], f32r)
                yT = ytp.tile([P, QPG, L], f32r)  # normalized attn out ^T
                nc.sync.dma_start(
                    wo_t[:], wo_d.rearrange("(h p) m -> p h m", p=P))

                for qc in range(NT):
                    q0 = qc * 512
                    nkb = 4 * qc + 4
                    for h in range(QPG):
                        y_ps = ps_y.tile([P, 512], f32, tag="y")
                        rs_ps = ps_rs.tile([P, 512], f32, tag="rs")
                        for kb in range(nkb):
                            c0 = max(0, kb * P - q0)   # col offset inside chunk
                            w = 512 - c0
                            st_ps = ps_st.tile([P, w], f32, tag="st")
                            nc.tensor.matmul(
                                st_ps[:], kT[:, kb * P:(kb + 1) * P],
                                qT[:, h, q0 + c0:q0 + 512],
                                start=True, stop=True)
                            if kb >= 4 * qc:  # diagonal block: causal mask
                                nc.vector.tensor_add(
                                    st_ps[:, 0:P], st_ps[:, 0:P], neg_t[:])
                            pt = ptp.tile([P, w], f32r, tag="pt")
                            nc.scalar.activation(
                                pt[:], st_ps[:],
                                mybir.ActivationFunctionType.Exp,
                                bias=kbias_t[:, kb:kb + 1], scale=SCALE)
                            nc.tensor.matmul(
                                y_ps[:, c0:512], v_sb[:, kb, :], pt[:],
                                start=(kb == 0), stop=(kb == nkb - 1))
                            nc.tensor.matmul(
                                rs_ps[:, c0:512], ones_t[:], pt[:],
                                start=(kb == 0), stop=(kb == nkb - 1))
                        rec = rcp.tile([P, 512], f32, tag="rc")
                        nc.vector.reciprocal(rec[:], rs_ps[:])
                        nc.vector.tensor_mul(
                            yT[:, h, q0:q0 + 512], y_ps[:], rec[:])

                    # out projection for this chunk's 4 token tiles
                    for tt in range(qc * 4, qc * 4 + 4):
                        for oc in range(4):
                            o_ps = ps_o.tile([P, 512], f32, tag="o")
                            for h in range(QPG):
                                nc.tensor.matmul(
                                    o_ps[:], yT[:, h, tt * P:(tt + 1) * P],
                                    wo_t[:, h, oc * 512:(oc + 1) * 512],
                                    start=(h == 0), stop=(h == QPG - 1))
                            ot = otp.tile([P, 512], f32, tag="ot")
                            nc.vector.tensor_copy(ot[:], o_ps[:])
                            nc.sync.dma_start(
                                out_d[tt * P:(tt + 1) * P, oc * 512:(oc + 1) * 512],
                                ot[:])

    nc.compile()
    return nc


def _host_consts(attention_mask):
    half = HD // 2
    inv_freq = (1.0 / (ROPE_THETA ** (np.arange(half, dtype=np.float32) / half))
                ).astype(np.float32)
    pos = np.arange(L, dtype=np.float32)
    freqs = pos[None, :] * inv_freq[:, None]          # [64, L]
    cos = np.cos(freqs).astype(np.float32)
    sin = np.sin(freqs).astype(np.float32)
    cos2 = np.ascontiguousarray(np.concatenate([cos, cos], axis=0))
    sin2 = np.ascontiguousarray(np.concatenate([sin, sin], axis=0))
    k_idx = np.arange(P)[:, None]
    q_idx = np.arange(P)[None, :]
    negmask = np.where(k_idx <= q_idx, 0.0, -1e4).astype(np.float32)
    onesm = np.ones((P, P), np.float32)
    ident = np.eye(P, dtype=np.float32)
    # key mask bias per batch: [P, TT] with partition p, col t -> key t*128+p
    kbias = []
    for b in range(B):
        m = attention_mask[b].astype(np.float32)      # [L]
        bias = np.where(m > 0, 0.0, -1e4).astype(np.float32)
        kbias.append(np.ascontiguousarray(bias.reshape(TT, P).T))
    return cos2, sin2, negmask, onesm, ident, kbias


def kernel(x, Wq, Wc, Wk, Wv, Wo, attention_mask):
    x = np.asarray(x, dtype=np.float32)
    Wq = np.asarray(Wq, dtype=np.float32)
    Wc = np.asarray(Wc, dtype=np.float32)
    Wk = np.asarray(Wk, dtype=np.float32)
    Wv = np.asarray(Wv, dtype=np.float32)
    Wo = np.asarray(Wo, dtype=np.float32)
    attention_mask = np.asarray(attention_mask)

    if "nc" not in _CACHE:
        _CACHE["nc"] = _build()
    nc = _CACHE["nc"]

    cos2, sin2, negmask, onesm, ident, kbias = _host_consts(attention_mask)
    xT = [np.ascontiguousarray(x[b].T) for b in range(B)]

    in_maps = []
    for core in range(8):
        b, g = core // QPG, core % QPG
        in_maps.append({
            "xT": xT[b],
            "wq": np.ascontiguousarray(Wq[:, g * QPG * HD:(g + 1) * QPG * HD]),
            "wc": Wc,
            "wk": np.ascontiguousarray(Wk[:, g * HD:(g + 1) * HD]),
            "wv": np.ascontiguousarray(Wv[:, g * HD:(g + 1) * HD]),
            "wo": np.ascontiguousarray(Wo[g * QPG * HD:(g + 1) * QPG * HD, :]),
            "cos2": cos2, "sin2": sin2, "negmask": negmask,
            "onesm": onesm, "ident": ident, "keybias": kbias[b],
        })

    res = run_bass_kernel_spmd(nc, in_maps, core_ids=list(range(8)))
    out = np.zeros((B, L, HID), dtype=np.float32)
    for core in range(8):
        out[core // QPG] += res.results[core]["out"]
    return out


# revision 21
# speedup vs baseline: 1.4003x; 1.4003x over previous
"""Causal self-attention MLA (GQA, latent kv) kernel for 8 Trainium2 cores.

Sharding: the 8 cores map to (batch b, kv-group g) pairs: core = b*4 + g.
Each core computes, for its batch and its kv head (4 q-heads):
  qT = Wq_g^T x^T (rope)
  kT = (Wc Wk_g)^T x^T (rope),  vT = (Wc Wv_g)^T x^T   [latent proj fused on
      host: exact in real arithmetic, differences at the f32r noise level]
  flash attention entirely in the transposed domain:
    ST[k,q] = kT^T qT  (per 128-k-block, causal blocks only)
    PT = exp(SCALE*ST + keybias)      (no max subtraction; logits ~N(0,1))
    yT[d,q] += v[kb]^T PT             (moving = PT -> no transposes anywhere)
    rs[*,q] += ones^T PT              (rowsum replicated across partitions)
    yTn = yT * 1/rs
  out_partial = yTn^T Wo_g  (row-parallel out proj)
Host sums the 4 partials per batch (free w.r.t. HW time).

Attention for query chunk qc is emitted right after projection chunk qc, so
the PE never drains at a phase boundary. All matmuls run in float32r
(TF32-like, full PE rate for moving dim >= 256); accumulation is fp32 in
PSUM. End-to-end rel err ~3e-4.
"""
import numpy as np

import concourse.bacc as bacc
import concourse.mybir as mybir
import concourse.tile as tile
from concourse.bass_utils import run_bass_kernel_spmd

B, L, HID = 2, 2048, 2048
NH, NKV, HD = 16, 4, 128
LAT = 512
QPG = NH // NKV            # q heads per kv group = 4
SCALE = float(HD) ** -0.5
ROPE_THETA = 10000.0
P = 128
NT = L // 512              # 4 token chunks of 512
KT = HID // P              # 16 contraction tiles
TT = L // P                # 16 token tiles of 128

dt = mybir.dt
f32, f32r = dt.float32, dt.float32r

_CACHE = {}


def _build():
    nc = bacc.Bacc("TRN2", target_bir_lowering=False, debug=False)

    # weight tensors arrive host-pre-transposed into SBUF layout
    xT_d = nc.dram_tensor("xT", [HID, L], f32r, kind="ExternalInput")
    wq_d = nc.dram_tensor("wq", [P, KT, QPG * HD], f32r, kind="ExternalInput")
    wck_d = nc.dram_tensor("wck", [P, KT, HD], f32r, kind="ExternalInput")
    wcv_d = nc.dram_tensor("wcv", [P, KT, HD], f32r, kind="ExternalInput")
    wo_d = nc.dram_tensor("wo", [P, QPG, HID], f32r, kind="ExternalInput")
    cos_d = nc.dram_tensor("cos2", [P, L], f32r, kind="ExternalInput")
    sin_d = nc.dram_tensor("sin2", [P, L], f32r, kind="ExternalInput")
    neg_d = nc.dram_tensor("negmask", [P, P], f32, kind="ExternalInput")
    ones_d = nc.dram_tensor("onesm", [P, P], f32r, kind="ExternalInput")
    idn_d = nc.dram_tensor("ident", [P, P], f32r, kind="ExternalInput")
    kb_d = nc.dram_tensor("keybias", [P, TT], f32, kind="ExternalInput")
    out_d = nc.dram_tensor("out", [L, HID], f32, kind="ExternalOutput")

    with tile.TileContext(nc) as tc:
        with tc.tile_pool(name="consts", bufs=1) as cp, \
             tc.tile_pool(name="qt", bufs=1) as qtp, \
             tc.tile_pool(name="kt", bufs=1) as ktp, \
             tc.tile_pool(name="vnat", bufs=1) as vnp, \
             tc.tile_pool(name="yt", bufs=1) as ytp:

            # consts through the idle GPSIMD software-DGE queue so the SP
            # queue starts streaming weights/activations immediately
            cos_t = cp.tile([P, L], f32r)
            sin_t = cp.tile([P, L], f32r)
            neg_t = cp.tile([P, P], f32)
            ones_t = cp.tile([P, P], f32r)
            idn_t = cp.tile([P, P], f32r)
            kbias_t = cp.tile([P, TT], f32)
            nc.gpsimd.dma_start(cos_t[:], cos_d[:])
            nc.gpsimd.dma_start(sin_t[:], sin_d[:])
            nc.gpsimd.dma_start(neg_t[:], neg_d[:])
            nc.gpsimd.dma_start(ones_t[:], ones_d[:])
            nc.gpsimd.dma_start(idn_t[:], idn_d[:])
            nc.gpsimd.dma_start(kbias_t[:], kb_d[:])

            qT = qtp.tile([P, QPG, L], f32r)     # per-head qT, roped in place
            kT = ktp.tile([P, L], f32r)          # kv-group kT, roped in place
            v_sb = vnp.tile([P, TT, HD], f32r)   # v natural [k, tile, d]
            yT = ytp.tile([P, QPG, L], f32r)     # normalized attention out ^T

            # unified PSUM pool: projection + attention share slots
            ps_cm = tc.tile_pool(name="ps", bufs=8, space="PSUM")
            ps = ps_cm.__enter__()

            with tc.tile_pool(name="wqc", bufs=1) as wp, \
                 tc.tile_pool(name="xt", bufs=6) as xp, \
                 tc.tile_pool(name="vt", bufs=2) as vtp, \
                 tc.tile_pool(name="rtmp", bufs=8) as rtp, \
                 tc.tile_pool(name="pt", bufs=4) as ptp, \
                 tc.tile_pool(name="rc", bufs=2) as rcp:

                wq_t = wp.tile([P, KT, QPG * HD], f32r)
                wck_t = wp.tile([P, KT, HD], f32r)
                wcv_t = wp.tile([P, KT, HD], f32r)

                def rope_chunk(dst, t):
                    """In-place rope of dst[:, t*512:(t+1)*512] (f32r)."""
                    c0, c1 = t * 512, (t + 1) * 512
                    t1c = rtp.tile([64, 512], f32, tag="rt")
                    t1s = rtp.tile([64, 512], f32, tag="rt")
                    t2c = rtp.tile([64, 512], f32, tag="rt")
                    t2s = rtp.tile([64, 512], f32, tag="rt")
                    nc.vector.tensor_mul(t1c[:], dst[0:64, c0:c1], cos_t[0:64, c0:c1])
                    nc.vector.tensor_mul(t1s[:], dst[0:64, c0:c1], sin_t[0:64, c0:c1])
                    nc.vector.tensor_mul(t2c[:], dst[64:128, c0:c1], cos_t[64:128, c0:c1])
                    nc.vector.tensor_mul(t2s[:], dst[64:128, c0:c1], sin_t[64:128, c0:c1])
                    nc.vector.tensor_sub(dst[0:64, c0:c1], t1c[:], t2s[:])
                    nc.vector.tensor_add(dst[64:128, c0:c1], t2c[:], t1s[:])

                def proj_chunk(t):
                    c0, c1 = t * 512, (t + 1) * 512
                    qps = [ps.tile([P, 512], f32, tag="ps1", name=f"qps{t}_{i}")
                           for i in range(QPG)]
                    kps = ps.tile([P, 512], f32, tag="ps1", name=f"kps{t}")
                    vps = ps.tile([P, 512], f32, tag="ps1", name=f"vps{t}")
                    for kt in range(KT):
                        if t == 0:
                            # lazy weight pieces: first matmuls unblock after
                            # one 256KB piece instead of the whole 6MB
                            nc.sync.dma_start(wck_t[:, kt, :], wck_d[:, kt, :])
                            nc.sync.dma_start(wcv_t[:, kt, :], wcv_d[:, kt, :])
                            nc.sync.dma_start(wq_t[:, kt, :], wq_d[:, kt, :])
                        xt = xp.tile([P, 512], f32r, tag="xt")
                        nc.sync.dma_start(xt[:], xT_d[kt * P:(kt + 1) * P, c0:c1])
                        st, sp = (kt == 0), (kt == KT - 1)
                        nc.tensor.matmul(kps[:], wck_t[:, kt, :], xt[:],
                                         start=st, stop=sp)
                        nc.tensor.matmul(vps[:], wcv_t[:, kt, :], xt[:],
                                         start=st, stop=sp)
                        for h in range(QPG):
                            nc.tensor.matmul(
                                qps[h][:], wq_t[:, kt, h * HD:(h + 1) * HD],
                                xt[:], start=st, stop=sp)
                    # PSUM evacuations on ACT (else idle here); rope on DVE
                    nc.scalar.copy(kT[:, c0:c1], kps[:])
                    rope_chunk(kT, t)
                    vt = vtp.tile([P, 512], f32r, tag="vt")
                    nc.scalar.copy(vt[:], vps[:])
                    for s in range(4):
                        tp = ps.tile([P, P], f32r, tag="ps1", name=f"tp{t}_{s}")
                        nc.tensor.transpose(tp[:], vt[:, s * P:(s + 1) * P], idn_t[:])
                        nc.scalar.copy(v_sb[:, t * 4 + s, :], tp[:])
                    for h in range(QPG):
                        nc.scalar.copy(qT[:, h, c0:c1], qps[h][:])
                        rope_chunk(qT[:, h, :], t)

                def attn_chunk(qc):
                    q0 = qc * 512
                    nkb = 4 * qc + 4
                    for h in range(QPG):
                        y_ps = ps.tile([P, 512], f32, tag="ps1", name=f"yps{qc}_{h}")
                        rs_ps = ps.tile([P, 512], f32, tag="ps1", name=f"rsps{qc}_{h}")
                        for kb in range(nkb):
                            c0 = max(0, kb * P - q0)  # col offset inside chunk
                            w = 512 - c0
                            st_ps = ps.tile([P, w], f32, tag="ps1",
                                            name=f"stps{qc}_{h}_{kb}")
                            nc.tensor.matmul(
                                st_ps[:], kT[:, kb * P:(kb + 1) * P],
                                qT[:, h, q0 + c0:q0 + 512],
                                start=True, stop=True)
                            if kb >= 4 * qc:  # diagonal block: causal mask
                                nc.vector.tensor_add(
                                    st_ps[:, 0:P], st_ps[:, 0:P], neg_t[:])
                            pt = ptp.tile([P, w], f32r, tag="pt")
                            nc.scalar.activation(
                                pt[:], st_ps[:],
                                mybir.ActivationFunctionType.Exp,
                                bias=kbias_t[:, kb:kb + 1], scale=SCALE)
                            nc.tensor.matmul(
                                y_ps[:, c0:512], v_sb[:, kb, :], pt[:],
                                start=(kb == 0), stop=(kb == nkb - 1))
                            nc.tensor.matmul(
                                rs_ps[:, c0:512], ones_t[:], pt[:],
                                start=(kb == 0), stop=(kb == nkb - 1))
                        rec = rcp.tile([P, 512], f32, tag="rc")
                        nc.vector.reciprocal(rec[:], rs_ps[:])
                        nc.vector.tensor_mul(
                            yT[:, h, q0:q0 + 512], y_ps[:], rec[:])

                for t in range(NT):
                    proj_chunk(t)
                    attn_chunk(t)

            # ---------------- out projection ----------------
            with tc.tile_pool(name="wo", bufs=1) as wop, \
                 tc.tile_pool(name="ot", bufs=4) as otp:
                wo_t = wop.tile([P, QPG, HID], f32r)
                for h in range(QPG):
                    nc.sync.dma_start(wo_t[:, h, :], wo_d[:, h, :])
                for tt in range(TT):
                    for oc in range(4):
                        o_ps = ps.tile([P, 512], f32, tag="ps1",
                                       name=f"ops{tt}_{oc}")
                        for h in range(QPG):
                            nc.tensor.matmul(
                                o_ps[:], yT[:, h, tt * P:(tt + 1) * P],
                                wo_t[:, h, oc * 512:(oc + 1) * 512],
                                start=(h == 0), stop=(h == QPG - 1))
                        ot = otp.tile([P, 512], f32, tag="ot")
                        nc.vector.tensor_copy(ot[:], o_ps[:])
                        nc.sync.dma_start(
                            out_d[tt * P:(tt + 1) * P, oc * 512:(oc + 1) * 512],
                            ot[:])

            ps_cm.__exit__(None, None, None)

    nc.compile()
    return nc


def _host_consts(attention_mask):
    half = HD // 2
    inv_freq = (1.0 / (ROPE_THETA ** (np.arange(half, dtype=np.float32) / half))
                ).astype(np.float32)
    pos = np.arange(L, dtype=np.float32)
    freqs = pos[None, :] * inv_freq[:, None]          # [64, L]
    cos = np.cos(freqs).astype(np.float32)
    sin = np.sin(freqs).astype(np.float32)
    cos2 = np.ascontiguousarray(np.concatenate([cos, cos], axis=0))
    sin2 = np.ascontiguousarray(np.concatenate([sin, sin], axis=0))
    k_idx = np.arange(P)[:, None]
    q_idx = np.arange(P)[None, :]
    negmask = np.where(k_idx <= q_idx, 0.0, -1e4).astype(np.float32)
    onesm = np.ones((P, P), np.float32)
    ident = np.eye(P, dtype=np.float32)
    # key mask bias per batch: [P, TT] with partition p, col t -> key t*128+p
    kbias = []
    for b in range(B):
        m = attention_mask[b].astype(np.float32)      # [L]
        bias = np.where(m > 0, 0.0, -1e4).astype(np.float32)
        kbias.append(np.ascontiguousarray(bias.reshape(TT, P).T))
    return cos2, sin2, negmask, onesm, ident, kbias


def kernel(x, Wq, Wc, Wk, Wv, Wo, attention_mask):
    x = np.asarray(x, dtype=np.float32)
    Wq = np.asarray(Wq, dtype=np.float32)
    Wc = np.asarray(Wc, dtype=np.float32)
    Wk = np.asarray(Wk, dtype=np.float32)
    Wv = np.asarray(Wv, dtype=np.float32)
    Wo = np.asarray(Wo, dtype=np.float32)
    attention_mask = np.asarray(attention_mask)

    if "nc" not in _CACHE:
        _CACHE["nc"] = _build()
    nc = _CACHE["nc"]

    cos2, sin2, negmask, onesm, ident, kbias = _host_consts(attention_mask)
    xT = [np.ascontiguousarray(x[b].T) for b in range(B)]
    # fuse the latent projection on host (exact up to fp rounding)
    Wck = (Wc.astype(np.float64) @ Wk.astype(np.float64)).astype(np.float32)
    Wcv = (Wc.astype(np.float64) @ Wv.astype(np.float64)).astype(np.float32)

    def sb_layout(w, inner):  # [K, M] -> [P, K//P, M] partition-major
        return np.ascontiguousarray(
            w.reshape(-1, P, inner).transpose(1, 0, 2))

    in_maps = []
    for core in range(8):
        b, g = core // QPG, core % QPG
        in_maps.append({
            "xT": xT[b],
            "wq": sb_layout(Wq[:, g * QPG * HD:(g + 1) * QPG * HD], QPG * HD),
            "wck": sb_layout(Wck[:, g * HD:(g + 1) * HD], HD),
            "wcv": sb_layout(Wcv[:, g * HD:(g + 1) * HD], HD),
            "wo": sb_layout(Wo[g * QPG * HD:(g + 1) * QPG * HD, :], HID),
            "cos2": cos2, "sin2": sin2, "negmask": negmask,
            "onesm": onesm, "ident": ident, "keybias": kbias[b],
        })

    res = run_bass_kernel_spmd(nc, in_maps, core_ids=list(range(8)))
    out = np.zeros((B, L, HID), dtype=np.float32)
    for core in range(8):
        out[core // QPG] += res.results[core]["out"]
    return out
